# revision 22
# baseline (speedup 1.0000x reference)
"""Cross-attention Trainium2 kernel, 8-way head-sharded (tensor parallel).

Strategy (per spec sharding_hint): split the 16 heads across the 8 cores
(2 heads / core) by slicing Wq/Wk/Wv column-wise (rows of the [out,in]
weight) and Wo row-wise. Each core computes q/k/v projections for its
128-dim slice, the masked-softmax attention for its 2 heads, then the
full-C output projection for a 1/8 slice of the (B*N) rows after an
AllToAll that redistributes the per-core attention outputs from
head-sharded to row-sharded. Host concatenates the 8 row slices.

v2 (this file): activations are pre-transposed AND pre-cast to bf16 on
the host (x^T [C,BN], ctx^T [C,BM]); the key-padding mask is folded into
ctx^T on the host (masked keys' columns zeroed), so the kernel has no
on-chip x/ctx transposes and no mask multiplies at all. All matmuls run
bf16 x bf16 with fp32 PSUM accumulation (1 cycle/row on the PE, same as
f32r, but transposes/DMA are 1-2x cheaper and no <256-column penalty).
Softmax denominators come from an extra mask column appended to V inside
the same PE accumulation as E@V; their reciprocal uses the fast DVE
approx op (~5x faster than nc.vector.reciprocal). Attention-phase
emission interleaves the next batch's K/V projection chunks between
score/AV matmul pairs so the PE never drains while the Act engine
(exp) catches up. End-to-end relative error ~4e-3 (tolerance 2e-2).
"""
import sys
sys.path.insert(0, '/opt/trn_rl_repo')

import numpy as np
import ml_dtypes

B, N, M, C, H, D = 4, 512, 2048, 1024, 16, 64
R = 8               # cores
DL = C // R         # per-core q/k/v slice width (2 heads x 64)
SCALE = D ** -0.5
BN, BM = B * N, B * M
CC = C // 128       # contraction chunks
MT = M // 128       # m-tiles per batch
TM = B * MT         # global m-tiles (64)
P = 128

_cached = {}


def _build():
    import concourse.tile as tile
    from concourse import bacc, mybir
    from concourse.masks import make_identity
    from contextlib import ExitStack

    F32 = mybir.dt.float32
    F32R = mybir.dt.float32r
    BF16 = mybir.dt.bfloat16
    AF = mybir.ActivationFunctionType
    OP = mybir.AluOpType

    nc = bacc.Bacc("TRN2", target_bir_lowering=False, debug=False, num_devices=R)

    xT_d = nc.dram_tensor("xT", [C, BN], BF16, kind="ExternalInput").ap()
    cT_d = nc.dram_tensor("cT", [C, BM], BF16, kind="ExternalInput").ap()
    mcol_d = nc.dram_tensor("mcol", [P, TM, 2], BF16, kind="ExternalInput").ap()
    wqkv_d = nc.dram_tensor("wqkv", [C, 3, DL], BF16, kind="ExternalInput").ap()
    wo_d = nc.dram_tensor("wo", [C, C], BF16, kind="ExternalInput").ap()
    bqk_d = nc.dram_tensor("bqk", [DL, 2], F32, kind="ExternalInput").ap()
    bo_d = nc.dram_tensor("bo", [P, C], F32, kind="ExternalInput").ap()
    out_d = nc.dram_tensor("out", [2, P, C], BF16, kind="ExternalOutput").ap()

    with tile.TileContext(nc) as tc, ExitStack() as es:
        const = es.enter_context(tc.tile_pool(name="const", bufs=1))
        kt_pool = es.enter_context(tc.tile_pool(name="kt", bufs=8))
        vn_pool = es.enter_context(tc.tile_pool(name="vn", bufs=8))
        qt_pool = es.enter_context(tc.tile_pool(name="qt", bufs=4))
        av_pool = es.enter_context(tc.tile_pool(name="av", bufs=2))
        agp = es.enter_context(tc.tile_pool(name="agp", bufs=2))
        outp = es.enter_context(tc.tile_pool(name="outp", bufs=4))
        dram = es.enter_context(tc.tile_pool(name="dram", bufs=1, space="DRAM"))
        pst = es.enter_context(tc.tile_pool(name="pst", bufs=2, space="PSUM"))
        psp = es.enter_context(tc.tile_pool(name="psp", bufs=2, space="PSUM"))
        pss = es.enter_context(tc.tile_pool(name="pss", bufs=2, space="PSUM"))
        psa = es.enter_context(tc.tile_pool(name="psa", bufs=2, space="PSUM"))

        # ---- constants: one priority-ordered stream on the sync queue so
        # early ctx slabs are never starved by later/larger loads ----
        wqkv_t = const.tile([P, CC, 3, DL], BF16, tag="wqkv")
        nc.sync.dma_start(wqkv_t[:], wqkv_d.rearrange("(cc p) w d -> p cc w d", p=P))
        wq_t = wqkv_t[:, :, 0]
        wk_t = wqkv_t[:, :, 1]
        wv_t = wqkv_t[:, :, 2]
        bqk_t = const.tile([P, 2], F32, tag="bqk")
        nc.sync.dma_start(bqk_t[:], bqk_d[:])
        bq_t = bqk_t[:, 0:1]
        bk_t = bqk_t[:, 1:2]
        mcolt = const.tile([P, TM, 2], BF16, tag="mcol")
        nc.sync.dma_start(mcolt[:], mcol_d[:])
        bo_t = const.tile([P, C], F32, tag="bo")
        nc.scalar.dma_start(bo_t[:], bo_d[:])
        identf = const.tile([P, P], F32, tag="idf")
        make_identity(nc, identf[:])
        identr = const.tile([P, P], F32R, tag="idr")
        nc.vector.tensor_copy(identr[:], identf[:])
        ones_b = const.tile([1, 64], BF16, tag="ones")
        nc.gpsimd.memset(ones_b[:], 1.0)
        wo_t = const.tile([P, CC, C], BF16, tag="wo")

        es2 = ExitStack()
        xn_pool = es2.enter_context(tc.tile_pool(name="xn", bufs=2))
        cn_pool = es2.enter_context(tc.tile_pool(name="cn", bufs=4))
        vt_pool = es2.enter_context(tc.tile_pool(name="vt", bufs=2))
        e_pool = es2.enter_context(tc.tile_pool(name="e", bufs=6))
        nrm_pool = es2.enter_context(tc.tile_pool(name="nrm", bufs=4))

        kt_tiles = []
        vn_tiles = []
        qt_tiles = []
        av_tiles = []
        a2a_ins = []
        a2a_outs = []
        cslabs = {}

        # ---- DMA stream, consumption order: ctx0 ctx1 x0 ctx2 x1 ctx3 x2
        # x3 ctx4.. (barrier + wo_t wedge in behind the early slabs) ----
        xslabs = []

        def x_dma(b):
            xs = xn_pool.tile([P, CC, 512], BF16, tag="xs")
            nc.sync.dma_start(
                xs[:], xT_d[:, b * 512:(b + 1) * 512]
                .rearrange("(cc p) n -> p cc n", p=P))
            xslabs.append(xs)

        def ctx_dma(mc):
            cs = cn_pool.tile([P, CC, 512], BF16, tag="cs")
            nc.sync.dma_start(
                cs[:], cT_d[:, mc * 512:(mc + 1) * 512]
                .rearrange("(cc p) n -> p cc n", p=P))
            cslabs[mc] = cs

        ctx_dma(0)
        ctx_dma(1)
        x_dma(0)
        ctx_dma(2)
        ctx_dma(3)
        x_dma(1)

        bar_in = dram.tile([1, 4], F32, tag="barin")
        bar_out = dram.tile([R, 4], F32, tag="barout")
        nc.scalar.dma_start(bar_in[:], bqk_d.rearrange("d o -> o d")[0:1, 0:4])
        nc.gpsimd.collective_compute(
            "AllGather", OP.bypass, replica_groups=[list(range(R))],
            ins=[bar_in.opt()], outs=[bar_out.opt()])
        warm_in = dram.tile([R, P, P], BF16, tag="warmin")
        warm_out = dram.tile([R, P, P], BF16, tag="warmout")
        nc.gpsimd.collective_compute(
            "AllToAll", OP.bypass, replica_groups=[list(range(R))],
            ins=[warm_in.opt()], outs=[warm_out.opt()])

        def q_proj(b):
            pq = psp.tile([P, 512], F32, tag="p")
            for cc in range(CC):
                nc.tensor.matmul(pq[:], lhsT=wq_t[:, cc], rhs=xslabs[b][:, cc, :],
                                 start=(cc == 0), stop=(cc == CC - 1))
            qt = qt_pool.tile([P, 512], BF16, tag="qt")
            nc.scalar.activation(qt[:], pq[:], AF.Identity, bias=bq_t, scale=1.0)
            qt_tiles.append(qt)

        # ---- Phase B slab chunks (emitted inline or as attention filler) ----
        def slab_chunks(mc):
            def c_k():
                pk = psp.tile([P, 512], F32, tag="p", name=f"pk{mc}")
                for cc in range(CC):
                    nc.tensor.matmul(pk[:], lhsT=wk_t[:, cc],
                                     rhs=cslabs[mc][:, cc, :],
                                     start=(cc == 0), stop=(cc == CC - 1))
                kt = kt_pool.tile([P, 512], BF16, tag="kt")
                nc.scalar.activation(kt[:], pk[:], AF.Identity, bias=bk_t, scale=1.0)
                kt_tiles.append(kt)

            def c_v():
                pv = psp.tile([P, 512], F32, tag="p", name=f"pv{mc}")
                for cc in range(CC):
                    nc.tensor.matmul(pv[:], lhsT=wv_t[:, cc],
                                     rhs=cslabs[mc][:, cc, :],
                                     start=(cc == 0), stop=(cc == CC - 1))
                vt_sb = vt_pool.tile([P, 512], F32R, tag="vt")
                nc.vector.tensor_copy(vt_sb[:], pv[:])
                cslabs[mc] = None
                return vt_sb

            def c_t(vt_sb):
                pvt = pst.tile([P, 512], F32R, tag="t")
                for j in range(4):
                    nc.tensor.transpose(pvt[:, j * P:(j + 1) * P],
                                        vt_sb[:, j * P:(j + 1) * P], identr[:])
                vn = vn_pool.tile([P, 4, 2, 65], BF16, tag="vn")
                nc.vector.tensor_copy(
                    vn[:, :, :, 0:64],
                    pvt[:].rearrange("p (j a d) -> p j a d", j=4, a=2))
                nc.vector.tensor_copy(vn[:, :, :, 64:65],
                                      mcolt[:, mc * 4:(mc + 1) * 4, :])
                vn_tiles.append(vn)

            state = {}
            def chunk1(): state['v'] = None; c_k()
            def chunk2(): state['v'] = c_v()
            def chunk3(): c_t(state['v'])
            return [chunk1, chunk2, chunk3]

        # ---- attention per batch, with filler chunks interleaved ----
        def attention_batch(b, filler):
            pav = [psa.tile([P, 512], F32, tag="a", name=f"pav{_h}") for _h in range(2)]
            fi = 0
            for mt in range(MT):
                tm = b * MT + mt
                mc, j = tm // 4, tm % 4
                kt = kt_tiles[mc]
                vn = vn_tiles[mc]
                ps = [pss.tile([P, 512], F32, tag="s", name=f"ps{_h}") for _h in range(2)]
                for h in range(2):
                    nc.tensor.matmul(ps[h][:],
                                     lhsT=kt[h * 64:(h + 1) * 64, j * P:(j + 1) * P],
                                     rhs=qt_tiles[b][h * 64:(h + 1) * 64, :],
                                     start=True, stop=True)
                ee = []
                for h in range(2):
                    e = e_pool.tile([P, 512], BF16, tag="e")
                    nc.scalar.activation(e[:], ps[h][:], AF.Exp,
                                         bias=0.0, scale=float(SCALE))
                    ee.append(e)
                if fi < len(filler):
                    filler[fi]()
                    fi += 1
                first, last = (mt == 0), (mt == MT - 1)
                for h in range(2):
                    # lhsT = [V_h | mask]: rows 0:64 = (EV)^T, row 64 = denom
                    nc.tensor.matmul(pav[h][0:65, :], lhsT=vn[:, j, h, :],
                                     rhs=ee[h][:], start=first, stop=last)
            while fi < len(filler):
                filler[fi]()
                fi += 1
            avt = av_pool.tile([P, 512], BF16, tag="av")
            for h in range(2):
                den = nrm_pool.tile([1, 512], F32, tag="den")
                nc.vector.tensor_copy(den[:], pav[h][64:65, :])
                rec = nrm_pool.tile([1, 512], F32, tag="rec")
                nc.vector.reciprocal_approx_fast(rec[:], den[:])
                rec_b = nrm_pool.tile([1, 512], BF16, tag="recb")
                nc.vector.tensor_copy(rec_b[:], rec[:])
                pb = psp.tile([P, 512], F32, tag="p")
                nc.tensor.matmul(pb[0:64, :], lhsT=ones_b[:], rhs=rec_b[:],
                                 start=True, stop=True)
                bc = nrm_pool.tile([64, 512], F32, tag="bc")
                nc.vector.tensor_copy(bc[:], pb[0:64, :])
                nc.vector.scalar_tensor_tensor(
                    out=avt[h * 64:(h + 1) * 64, :], in0=pav[h][0:64, :],
                    scalar=1.0, in1=bc[:], op0=OP.mult, op1=OP.mult)
            av_tiles.append(avt)
            hh = b // 2
            if b % 2 == 0:
                a2a_in = dram.tile([R, P, P], BF16, name=f"a2ai{hh}")
                a2a_ins.append(a2a_in)
            else:
                a2a_in = a2a_ins[hh]
            # stage this batch's 4 chunks now (slot j gets batch 2hh+j//4)
            for j in range(R):
                if j // 4 != b % 2:
                    continue
                nc.scalar.dma_start(a2a_in[j, :, :],
                                    avt[:, (j % 4) * P:(j % 4 + 1) * P])
            if b % 2 == 1:
                a2a_out = dram.tile([R, P, P], BF16, name=f"a2ao{hh}")
                nc.gpsimd.collective_compute(
                    "AllToAll", OP.bypass, replica_groups=[list(range(R))],
                    ins=[a2a_in.opt()], outs=[a2a_out.opt()])
                a2a_outs.append(a2a_out)

        def wo_chunks(h):
            """Deferred behind the a2a so the collective completes off the
            critical path (in-order engine queues)."""
            state = {}

            def load():
                agt = agp.tile([P, CC, P], BF16, name=f"agt{h}", tag="ag")
                src_r = a2a_outs[h].rearrange("i p n -> p i n")
                nc.scalar.dma_start(agt[:, 0:4], src_r[:, 0:4])
                nc.scalar.dma_start(agt[:, 4:8], src_r[:, 4:8])
                state['agt'] = agt

            def ch_chunk(ch):
                agt = state['agt']
                po = psp.tile([P, 512], F32, tag="p", name=f"po{h}_{ch}")
                for cc in range(CC):
                    nc.tensor.matmul(po[:], lhsT=agt[:, cc, :],
                                     rhs=wo_t[:, cc, ch * 512:(ch + 1) * 512],
                                     start=(cc == 0), stop=(cc == CC - 1))
                ob = outp.tile([P, 512], BF16, tag="ob", name=f"ob{h}_{ch}")
                nc.vector.scalar_tensor_tensor(
                    out=ob[:], in0=po[:], scalar=1.0,
                    in1=bo_t[:, ch * 512:(ch + 1) * 512],
                    op0=OP.mult, op1=OP.add)
                nc.scalar.dma_start(out_d[h, :, ch * 512:(ch + 1) * 512], ob[:])

            return [load, lambda: ch_chunk(0), lambda: ch_chunk(1)]

        # slabs 0-3 inline (batch 0's keys), then attention batches with the
        # next batch's slab chunks (or wo_half(0)) as PE filler.
        # prologue: slabs 0-3 + q-projections, ctx dmas rolling 4 ahead
        chunks0 = slab_chunks(0)
        chunks1 = slab_chunks(1)
        chunks0[0](); chunks0[1](); chunks0[2]()
        ctx_dma(4)
        q_proj(0)
        chunks1[0](); chunks1[1](); chunks1[2]()
        ctx_dma(5)
        q_proj(1)
        for c in slab_chunks(2):
            c()
        x_dma(2)
        ctx_dma(6)
        q_proj(2)
        for c in slab_chunks(3):
            c()
        x_dma(3)
        ctx_dma(7)
        q_proj(3)
        wo_dmaed = False
        noop = lambda: None
        for b in range(B):
            if b < 3:
                filler = []
                nslab = 4 if b < 2 else 3
                for k in range(nslab):
                    mc = 4 * (b + 1) + k
                    cks = slab_chunks(mc)
                    def mk_tail(ck3, nxt):
                        def f():
                            ck3()
                            if nxt is not None:
                                ctx_dma(nxt)
                        return f
                    nxt = mc + 4 if mc + 4 < 16 else None
                    filler += [cks[0], cks[1], mk_tail(cks[2], nxt)]
            else:
                w0 = wo_chunks(0)
                s15 = slab_chunks(15)
                filler = [s15[0], s15[1], s15[2], noop, noop, noop,
                          w0[0], noop, w0[1], noop, w0[2]]
            attention_batch(b, filler)
            if b == 1 and not wo_dmaed:
                # wo arrives behind all ctx slabs, ahead of first use
                nc.sync.dma_start(wo_t[:], wo_d.rearrange("(cc p) c -> p cc c", p=P))
                wo_dmaed = True
        for c in wo_chunks(1):
            c()

        es2.close()

    nc.compile()
    return nc


def _get_nc():
    if "nc" not in _cached:
        _cached["nc"] = _build()
    return _cached["nc"]


def _prep_inputs(x, context, ctx_key_padding_mask, Wq, bq, Wk, bk, Wv, bv, Wo, bo):
    bf16 = ml_dtypes.bfloat16
    x = np.asarray(x, dtype=np.float32).reshape(BN, C)
    ctx = np.asarray(context, dtype=np.float32).reshape(BM, C)
    mask = np.asarray(ctx_key_padding_mask, dtype=np.int32)
    Wq = np.asarray(Wq, dtype=np.float32)
    Wk = np.asarray(Wk, dtype=np.float32)
    Wv = np.asarray(Wv, dtype=np.float32)
    Wo = np.asarray(Wo, dtype=np.float32)
    bq = np.asarray(bq, dtype=np.float32)
    bk = np.asarray(bk, dtype=np.float32)
    bv = np.asarray(bv, dtype=np.float32)
    bo = np.asarray(bo, dtype=np.float32)
    # bv folds through the (row-stochastic) attention and Wo exactly:
    # out = (attn + bv) @ Wo.T + bo = attn @ Wo.T + (bo + Wo @ bv)
    bo_eff = (bo.astype(np.float64) + Wo.astype(np.float64) @ bv.astype(np.float64)
              ).astype(np.float32)
    bo_bc = np.ascontiguousarray(np.broadcast_to(bo_eff, (P, C)))
    # key-padding mask folded into ctx^T: masked keys' columns become 0, so
    # v is masked for free; their k rows are bk-only but those E values are
    # multiplied by the zero mask column in both softmax sums.
    mf = (mask != 0).astype(np.float32).reshape(BM)      # [BM]
    xT = np.ascontiguousarray(x.T).astype(bf16)          # [C, BN]
    cT = np.ascontiguousarray((ctx * mf[:, None]).T).astype(bf16)  # [C, BM]
    # mcol[p, tm, h] = mask value for key tm*128+p (same for both heads)
    mcol = np.ascontiguousarray(
        np.broadcast_to(mf.reshape(TM, P).T[:, :, None], (P, TM, 2))).astype(bf16)
    wo_full = np.ascontiguousarray(Wo.T).astype(bf16)
    in_maps = []
    for r in range(R):
        sl = slice(r * DL, (r + 1) * DL)
        wqkv = np.stack([Wq[sl, :].T, Wk[sl, :].T, Wv[sl, :].T], axis=1)
        bqk = np.stack([bq[sl], bk[sl]], axis=1)
        in_maps.append({
            "xT": xT, "cT": cT, "mcol": mcol,
            "wqkv": np.ascontiguousarray(wqkv).astype(bf16),
            "wo": wo_full,
            "bqk": np.ascontiguousarray(bqk, dtype=np.float32),
            "bo": bo_bc,
        })
    return in_maps


def _run(in_maps, **kwargs):
    from concourse.bass_utils import run_bass_kernel_spmd
    nc = _get_nc()
    return run_bass_kernel_spmd(nc, in_maps, list(range(R)), **kwargs)


def kernel(x, context, ctx_key_padding_mask, Wq, bq, Wk, bk, Wv, bv, Wo, bo):
    in_maps = _prep_inputs(x, context, ctx_key_padding_mask,
                           Wq, bq, Wk, bk, Wv, bv, Wo, bo)
    res = _run(in_maps).results
    out = np.empty((BN, C), dtype=np.float32)
    for r in range(R):
        o = res[r]["out"]          # [2, 128, C]: half h -> batch 2h + r//4,
        for h in range(2):         # rows (r%4)*128 ...
            b = 2 * h + r // 4
            row = b * N + (r % 4) * P
            out[row:row + P] = o[h]
    return np.ascontiguousarray(out.reshape(B, N, C))


# revision 23
# speedup vs baseline: 1.0560x; 1.0560x over previous
"""Cross-attention Trainium2 kernel, 8-way head-sharded (tensor parallel).

Strategy (per spec sharding_hint): split the 16 heads across the 8 cores
(2 heads / core) by slicing Wq/Wk/Wv column-wise (rows of the [out,in]
weight) and Wo row-wise. Each core computes q/k/v projections for its
128-dim slice, the masked-softmax attention for its 2 heads, then the
full-C output projection for a 1/8 slice of the (B*N) rows after an
AllToAll that redistributes the per-core attention outputs from
head-sharded to row-sharded. Host concatenates the 8 row slices.

v2 (this file): activations are pre-transposed AND pre-cast to bf16 on
the host (x^T [C,BN], ctx^T [C,BM]); the key-padding mask is folded into
ctx^T on the host (masked keys' columns zeroed), so the kernel has no
on-chip x/ctx transposes and no mask multiplies at all. All matmuls run
bf16 x bf16 with fp32 PSUM accumulation (1 cycle/row on the PE, same as
f32r, but transposes/DMA are 1-2x cheaper and no <256-column penalty).
Softmax denominators come from an extra mask column appended to V inside
the same PE accumulation as E@V; their reciprocal uses the fast DVE
approx op (~5x faster than nc.vector.reciprocal). Attention-phase
emission interleaves the next batch's K/V projection chunks between
score/AV matmul pairs so the PE never drains while the Act engine
(exp) catches up. End-to-end relative error ~4e-3 (tolerance 2e-2).
"""
import sys
sys.path.insert(0, '/opt/trn_rl_repo')

import numpy as np
import ml_dtypes

B, N, M, C, H, D = 4, 512, 2048, 1024, 16, 64
R = 8               # cores
DL = C // R         # per-core q/k/v slice width (2 heads x 64)
SCALE = D ** -0.5
BN, BM = B * N, B * M
CC = C // 128       # contraction chunks
MT = M // 128       # m-tiles per batch
TM = B * MT         # global m-tiles (64)
P = 128

_cached = {}


def _build():
    import concourse.tile as tile
    from concourse import bacc, mybir
    from concourse.masks import make_identity
    from contextlib import ExitStack

    F32 = mybir.dt.float32
    F32R = mybir.dt.float32r
    BF16 = mybir.dt.bfloat16
    AF = mybir.ActivationFunctionType
    OP = mybir.AluOpType

    nc = bacc.Bacc("TRN2", target_bir_lowering=False, debug=False, num_devices=R)

    xT_d = nc.dram_tensor("xT", [C, BN], BF16, kind="ExternalInput").ap()
    cT_d = nc.dram_tensor("cT", [C, BM], BF16, kind="ExternalInput").ap()
    mcol_d = nc.dram_tensor("mcol", [P, TM, 2], BF16, kind="ExternalInput").ap()
    wqkv_d = nc.dram_tensor("wqkv", [C, 3, DL], BF16, kind="ExternalInput").ap()
    wo_d = nc.dram_tensor("wo", [C, C], BF16, kind="ExternalInput").ap()
    bqk_d = nc.dram_tensor("bqk", [DL, 2], F32, kind="ExternalInput").ap()
    bo_d = nc.dram_tensor("bo", [P, C], F32, kind="ExternalInput").ap()
    out_d = nc.dram_tensor("out", [2, P, C], BF16, kind="ExternalOutput").ap()

    with tile.TileContext(nc) as tc, ExitStack() as es:
        const = es.enter_context(tc.tile_pool(name="const", bufs=1))
        kt_pool = es.enter_context(tc.tile_pool(name="kt", bufs=8))
        vn_pool = es.enter_context(tc.tile_pool(name="vn", bufs=8))
        qt_pool = es.enter_context(tc.tile_pool(name="qt", bufs=4))
        av_pool = es.enter_context(tc.tile_pool(name="av", bufs=2))
        agp = es.enter_context(tc.tile_pool(name="agp", bufs=2))
        outp = es.enter_context(tc.tile_pool(name="outp", bufs=4))
        dram = es.enter_context(tc.tile_pool(name="dram", bufs=1, space="DRAM"))
        pst = es.enter_context(tc.tile_pool(name="pst", bufs=2, space="PSUM"))
        psp = es.enter_context(tc.tile_pool(name="psp", bufs=2, space="PSUM"))
        pss = es.enter_context(tc.tile_pool(name="pss", bufs=2, space="PSUM"))
        psa = es.enter_context(tc.tile_pool(name="psa", bufs=2, space="PSUM"))

        # ---- constants: one priority-ordered stream on the sync queue so
        # early ctx slabs are never starved by later/larger loads ----
        wqkv_t = const.tile([P, CC, 3, DL], BF16, tag="wqkv")
        nc.sync.dma_start(wqkv_t[:], wqkv_d.rearrange("(cc p) w d -> p cc w d", p=P))
        wq_t = wqkv_t[:, :, 0]
        wk_t = wqkv_t[:, :, 1]
        wv_t = wqkv_t[:, :, 2]
        bqk_t = const.tile([P, 2], F32, tag="bqk")
        nc.sync.dma_start(bqk_t[:], bqk_d[:])
        bq_t = bqk_t[:, 0:1]
        bk_t = bqk_t[:, 1:2]
        mcolt = const.tile([P, TM, 2], BF16, tag="mcol")
        nc.sync.dma_start(mcolt[:], mcol_d[:])
        bo_t = const.tile([P, C], F32, tag="bo")
        nc.scalar.dma_start(bo_t[:], bo_d[:])
        identf = const.tile([P, P], F32, tag="idf")
        make_identity(nc, identf[:])
        identr = const.tile([P, P], F32R, tag="idr")
        nc.vector.tensor_copy(identr[:], identf[:])
        ones_b = const.tile([1, 64], BF16, tag="ones")
        nc.gpsimd.memset(ones_b[:], 1.0)
        wo_t = const.tile([P, CC, C], BF16, tag="wo")

        es2 = ExitStack()
        xn_pool = es2.enter_context(tc.tile_pool(name="xn", bufs=2))
        cn_pool = es2.enter_context(tc.tile_pool(name="cn", bufs=4))
        vt_pool = es2.enter_context(tc.tile_pool(name="vt", bufs=2))
        e_pool = es2.enter_context(tc.tile_pool(name="e", bufs=6))
        nrm_pool = es2.enter_context(tc.tile_pool(name="nrm", bufs=4))

        kt_tiles = []
        vn_tiles = []
        qt_tiles = []
        av_tiles = []
        a2a_ins = []
        a2a_outs = []
        cslabs = {}

        # ---- DMA stream, consumption order: ctx0 ctx1 x0 ctx2 x1 ctx3 x2
        # x3 ctx4.. (barrier + wo_t wedge in behind the early slabs) ----
        xslabs = []

        def x_dma(b):
            xs = xn_pool.tile([P, CC, 512], BF16, tag="xs")
            nc.sync.dma_start(
                xs[:], xT_d[:, b * 512:(b + 1) * 512]
                .rearrange("(cc p) n -> p cc n", p=P))
            xslabs.append(xs)

        def ctx_dma(mc):
            cs = cn_pool.tile([P, CC, 512], BF16, tag="cs")
            nc.sync.dma_start(
                cs[:], cT_d[:, mc * 512:(mc + 1) * 512]
                .rearrange("(cc p) n -> p cc n", p=P))
            cslabs[mc] = cs

        ctx_dma(0)
        ctx_dma(1)
        x_dma(0)
        ctx_dma(2)
        ctx_dma(3)
        x_dma(1)

        bar_in = dram.tile([1, 4], F32, tag="barin")
        bar_out = dram.tile([R, 4], F32, tag="barout")
        nc.scalar.dma_start(bar_in[:], bqk_d.rearrange("d o -> o d")[0:1, 0:4])
        nc.gpsimd.collective_compute(
            "AllGather", OP.bypass, replica_groups=[list(range(R))],
            ins=[bar_in.opt()], outs=[bar_out.opt()])
        def q_proj(b):
            pq = psp.tile([P, 512], F32, tag="p")
            for cc in range(CC):
                nc.tensor.matmul(pq[:], lhsT=wq_t[:, cc], rhs=xslabs[b][:, cc, :],
                                 start=(cc == 0), stop=(cc == CC - 1))
            qt = qt_pool.tile([P, 512], BF16, tag="qt")
            nc.scalar.activation(qt[:], pq[:], AF.Identity, bias=bq_t, scale=1.0)
            qt_tiles.append(qt)

        # ---- Phase B slab chunks (emitted inline or as attention filler) ----
        def slab_chunks(mc):
            def c_k():
                pk = psp.tile([P, 512], F32, tag="p", name=f"pk{mc}")
                for cc in range(CC):
                    nc.tensor.matmul(pk[:], lhsT=wk_t[:, cc],
                                     rhs=cslabs[mc][:, cc, :],
                                     start=(cc == 0), stop=(cc == CC - 1))
                kt = kt_pool.tile([P, 512], BF16, tag="kt")
                nc.scalar.activation(kt[:], pk[:], AF.Identity, bias=bk_t, scale=1.0)
                kt_tiles.append(kt)

            def c_v():
                pv = psp.tile([P, 512], F32, tag="p", name=f"pv{mc}")
                for cc in range(CC):
                    nc.tensor.matmul(pv[:], lhsT=wv_t[:, cc],
                                     rhs=cslabs[mc][:, cc, :],
                                     start=(cc == 0), stop=(cc == CC - 1))
                vt_sb = vt_pool.tile([P, 512], F32R, tag="vt")
                nc.vector.tensor_copy(vt_sb[:], pv[:])
                cslabs[mc] = None
                return vt_sb

            def c_t(vt_sb):
                pvt = pst.tile([P, 512], F32R, tag="t")
                for j in range(4):
                    nc.tensor.transpose(pvt[:, j * P:(j + 1) * P],
                                        vt_sb[:, j * P:(j + 1) * P], identr[:])
                vn = vn_pool.tile([P, 4, 2, 65], BF16, tag="vn")
                nc.vector.tensor_copy(
                    vn[:, :, :, 0:64],
                    pvt[:].rearrange("p (j a d) -> p j a d", j=4, a=2))
                nc.vector.tensor_copy(vn[:, :, :, 64:65],
                                      mcolt[:, mc * 4:(mc + 1) * 4, :])
                vn_tiles.append(vn)

            state = {}
            def chunk1(): state['v'] = None; c_k()
            def chunk2(): state['v'] = c_v()
            def chunk3(): c_t(state['v'])
            return [chunk1, chunk2, chunk3]

        # ---- attention per batch, with filler chunks interleaved ----
        def attention_batch(b, filler):
            pav = [psa.tile([P, 512], F32, tag="a", name=f"pav{_h}") for _h in range(2)]
            fi = 0
            for mt in range(MT):
                tm = b * MT + mt
                mc, j = tm // 4, tm % 4
                kt = kt_tiles[mc]
                vn = vn_tiles[mc]
                ps = [pss.tile([P, 512], F32, tag="s", name=f"ps{_h}") for _h in range(2)]
                for h in range(2):
                    nc.tensor.matmul(ps[h][:],
                                     lhsT=kt[h * 64:(h + 1) * 64, j * P:(j + 1) * P],
                                     rhs=qt_tiles[b][h * 64:(h + 1) * 64, :],
                                     start=True, stop=True)
                ee = []
                for h in range(2):
                    e = e_pool.tile([P, 512], BF16, tag="e")
                    nc.scalar.activation(e[:], ps[h][:], AF.Exp,
                                         bias=0.0, scale=float(SCALE))
                    ee.append(e)
                if fi < len(filler):
                    filler[fi]()
                    fi += 1
                first, last = (mt == 0), (mt == MT - 1)
                for h in range(2):
                    # lhsT = [V_h | mask]: rows 0:64 = (EV)^T, row 64 = denom
                    nc.tensor.matmul(pav[h][0:65, :], lhsT=vn[:, j, h, :],
                                     rhs=ee[h][:], start=first, stop=last)
            while fi < len(filler):
                filler[fi]()
                fi += 1
            avt = av_pool.tile([P, 512], BF16, tag="av")
            for h in range(2):
                den = nrm_pool.tile([1, 512], F32, tag="den")
                nc.vector.tensor_copy(den[:], pav[h][64:65, :])
                rec = nrm_pool.tile([1, 512], F32, tag="rec")
                nc.vector.reciprocal_approx_fast(rec[:], den[:])
                rec_b = nrm_pool.tile([1, 512], BF16, tag="recb")
                nc.vector.tensor_copy(rec_b[:], rec[:])
                pb = psp.tile([P, 512], F32, tag="p")
                nc.tensor.matmul(pb[0:64, :], lhsT=ones_b[:], rhs=rec_b[:],
                                 start=True, stop=True)
                bc = nrm_pool.tile([64, 512], F32, tag="bc")
                nc.vector.tensor_copy(bc[:], pb[0:64, :])
                nc.vector.scalar_tensor_tensor(
                    out=avt[h * 64:(h + 1) * 64, :], in0=pav[h][0:64, :],
                    scalar=1.0, in1=bc[:], op0=OP.mult, op1=OP.mult)
            av_tiles.append(avt)
            hh = b // 2
            if b % 2 == 0:
                a2a_in = dram.tile([R, P, P], BF16, name=f"a2ai{hh}")
                a2a_ins.append(a2a_in)
            else:
                a2a_in = a2a_ins[hh]
            # stage this batch's 4 chunks now (slot j gets batch 2hh+j//4)
            for j in range(R):
                if j // 4 != b % 2:
                    continue
                nc.scalar.dma_start(a2a_in[j, :, :],
                                    avt[:, (j % 4) * P:(j % 4 + 1) * P])
            if b % 2 == 1:
                a2a_out = dram.tile([R, P, P], BF16, name=f"a2ao{hh}")
                nc.gpsimd.collective_compute(
                    "AllToAll", OP.bypass, replica_groups=[list(range(R))],
                    ins=[a2a_in.opt()], outs=[a2a_out.opt()])
                a2a_outs.append(a2a_out)

        def wo_chunks(h):
            """Deferred behind the a2a so the collective completes off the
            critical path (in-order engine queues)."""
            state = {}

            def load():
                agt = agp.tile([P, CC, P], BF16, name=f"agt{h}", tag="ag")
                src_r = a2a_outs[h].rearrange("i p n -> p i n")
                nc.scalar.dma_start(agt[:, 0:4], src_r[:, 0:4])
                nc.scalar.dma_start(agt[:, 4:8], src_r[:, 4:8])
                state['agt'] = agt

            def ch_chunk(ch):
                agt = state['agt']
                po = psp.tile([P, 512], F32, tag="p", name=f"po{h}_{ch}")
                for cc in range(CC):
                    nc.tensor.matmul(po[:], lhsT=agt[:, cc, :],
                                     rhs=wo_t[:, cc, ch * 512:(ch + 1) * 512],
                                     start=(cc == 0), stop=(cc == CC - 1))
                ob = outp.tile([P, 512], BF16, tag="ob", name=f"ob{h}_{ch}")
                nc.vector.scalar_tensor_tensor(
                    out=ob[:], in0=po[:], scalar=1.0,
                    in1=bo_t[:, ch * 512:(ch + 1) * 512],
                    op0=OP.mult, op1=OP.add)
                nc.scalar.dma_start(out_d[h, :, ch * 512:(ch + 1) * 512], ob[:])

            return [load, lambda: ch_chunk(0), lambda: ch_chunk(1)]

        # slabs 0-3 inline (batch 0's keys), then attention batches with the
        # next batch's slab chunks (or wo_half(0)) as PE filler.
        # prologue: slabs 0-3 + q-projections, ctx dmas rolling 4 ahead
        chunks0 = slab_chunks(0)
        chunks1 = slab_chunks(1)
        chunks0[0](); chunks0[1](); chunks0[2]()
        ctx_dma(4)
        q_proj(0)
        chunks1[0](); chunks1[1](); chunks1[2]()
        ctx_dma(5)
        q_proj(1)
        for c in slab_chunks(2):
            c()
        x_dma(2)
        ctx_dma(6)
        q_proj(2)
        for c in slab_chunks(3):
            c()
        x_dma(3)
        ctx_dma(7)
        q_proj(3)
        wo_dmaed = False
        noop = lambda: None
        for b in range(B):
            if b < 3:
                filler = []
                nslab = 4 if b < 2 else 3
                for k in range(nslab):
                    mc = 4 * (b + 1) + k
                    cks = slab_chunks(mc)
                    def mk_tail(ck3, nxt):
                        def f():
                            ck3()
                            if nxt is not None:
                                ctx_dma(nxt)
                        return f
                    nxt = mc + 4 if mc + 4 < 16 else None
                    filler += [cks[0], cks[1], mk_tail(cks[2], nxt)]
            else:
                w0 = wo_chunks(0)
                s15 = slab_chunks(15)
                filler = [s15[0], s15[1], s15[2], noop, noop, noop,
                          w0[0], noop, w0[1], noop, w0[2]]
            attention_batch(b, filler)
            if b == 2 and not wo_dmaed:
                # wo lands between the two AllToAlls (collectives bounce
                # through HBM; avoid competing with them for bandwidth)
                nc.sync.dma_start(wo_t[:], wo_d.rearrange("(cc p) c -> p cc c", p=P))
                wo_dmaed = True
        for c in wo_chunks(1):
            c()

        es2.close()

    nc.compile()
    return nc


def _get_nc():
    if "nc" not in _cached:
        _cached["nc"] = _build()
    return _cached["nc"]


def _prep_inputs(x, context, ctx_key_padding_mask, Wq, bq, Wk, bk, Wv, bv, Wo, bo):
    bf16 = ml_dtypes.bfloat16
    x = np.asarray(x, dtype=np.float32).reshape(BN, C)
    ctx = np.asarray(context, dtype=np.float32).reshape(BM, C)
    mask = np.asarray(ctx_key_padding_mask, dtype=np.int32)
    Wq = np.asarray(Wq, dtype=np.float32)
    Wk = np.asarray(Wk, dtype=np.float32)
    Wv = np.asarray(Wv, dtype=np.float32)
    Wo = np.asarray(Wo, dtype=np.float32)
    bq = np.asarray(bq, dtype=np.float32)
    bk = np.asarray(bk, dtype=np.float32)
    bv = np.asarray(bv, dtype=np.float32)
    bo = np.asarray(bo, dtype=np.float32)
    # bv folds through the (row-stochastic) attention and Wo exactly:
    # out = (attn + bv) @ Wo.T + bo = attn @ Wo.T + (bo + Wo @ bv)
    bo_eff = (bo.astype(np.float64) + Wo.astype(np.float64) @ bv.astype(np.float64)
              ).astype(np.float32)
    bo_bc = np.ascontiguousarray(np.broadcast_to(bo_eff, (P, C)))
    # key-padding mask folded into ctx^T: masked keys' columns become 0, so
    # v is masked for free; their k rows are bk-only but those E values are
    # multiplied by the zero mask column in both softmax sums.
    mf = (mask != 0).astype(np.float32).reshape(BM)      # [BM]
    xT = np.ascontiguousarray(x.T).astype(bf16)          # [C, BN]
    cT = np.ascontiguousarray((ctx * mf[:, None]).T).astype(bf16)  # [C, BM]
    # mcol[p, tm, h] = mask value for key tm*128+p (same for both heads)
    mcol = np.ascontiguousarray(
        np.broadcast_to(mf.reshape(TM, P).T[:, :, None], (P, TM, 2))).astype(bf16)
    wo_full = np.ascontiguousarray(Wo.T).astype(bf16)
    in_maps = []
    for r in range(R):
        sl = slice(r * DL, (r + 1) * DL)
        wqkv = np.stack([Wq[sl, :].T, Wk[sl, :].T, Wv[sl, :].T], axis=1)
        bqk = np.stack([bq[sl], bk[sl]], axis=1)
        in_maps.append({
            "xT": xT, "cT": cT, "mcol": mcol,
            "wqkv": np.ascontiguousarray(wqkv).astype(bf16),
            "wo": wo_full,
            "bqk": np.ascontiguousarray(bqk, dtype=np.float32),
            "bo": bo_bc,
        })
    return in_maps


def _run(in_maps, **kwargs):
    from concourse.bass_utils import run_bass_kernel_spmd
    nc = _get_nc()
    return run_bass_kernel_spmd(nc, in_maps, list(range(R)), **kwargs)


def kernel(x, context, ctx_key_padding_mask, Wq, bq, Wk, bk, Wv, bv, Wo, bo):
    in_maps = _prep_inputs(x, context, ctx_key_padding_mask,
                           Wq, bq, Wk, bk, Wv, bv, Wo, bo)
    res = _run(in_maps).results
    out = np.empty((BN, C), dtype=np.float32)
    for r in range(R):
        o = res[r]["out"]          # [2, 128, C]: half h -> batch 2h + r//4,
        for h in range(2):         # rows (r%4)*128 ...
            b = 2 * h + r // 4
            row = b * N + (r % 4) * P
            out[row:row + P] = o[h]
    return np.ascontiguousarray(out.reshape(B, N, C))


# revision 24
# speedup vs baseline: 1.0980x; 1.0398x over previous
"""Cross-attention Trainium2 kernel, 8-way head-sharded (tensor parallel).

Strategy (per spec sharding_hint): split the 16 heads across the 8 cores
(2 heads / core) by slicing Wq/Wk/Wv column-wise (rows of the [out,in]
weight) and Wo row-wise. Each core computes q/k/v projections for its
128-dim slice, the masked-softmax attention for its 2 heads, then the
full-C output projection for a 1/8 slice of the (B*N) rows after an
AllToAll that redistributes the per-core attention outputs from
head-sharded to row-sharded. Host concatenates the 8 row slices.

v2 (this file): activations are pre-transposed AND pre-cast to bf16 on
the host (x^T [C,BN], ctx^T [C,BM]); the key-padding mask is folded into
ctx^T on the host (masked keys' columns zeroed), so the kernel has no
on-chip x/ctx transposes and no mask multiplies at all. All matmuls run
bf16 x bf16 with fp32 PSUM accumulation (1 cycle/row on the PE, same as
f32r, but transposes/DMA are 1-2x cheaper and no <256-column penalty).
Softmax denominators come from an extra mask column appended to V inside
the same PE accumulation as E@V; their reciprocal uses the fast DVE
approx op (~5x faster than nc.vector.reciprocal). Attention-phase
emission interleaves the next batch's K/V projection chunks between
score/AV matmul pairs so the PE never drains while the Act engine
(exp) catches up. End-to-end relative error ~4e-3 (tolerance 2e-2).
"""
import sys
sys.path.insert(0, '/opt/trn_rl_repo')

import numpy as np
import ml_dtypes

B, N, M, C, H, D = 4, 512, 2048, 1024, 16, 64
R = 8               # cores
DL = C // R         # per-core q/k/v slice width (2 heads x 64)
SCALE = D ** -0.5
BN, BM = B * N, B * M
CC = C // 128       # contraction chunks
MT = M // 128       # m-tiles per batch
TM = B * MT         # global m-tiles (64)
P = 128

_cached = {}


def _build():
    import concourse.tile as tile
    from concourse import bacc, mybir
    from concourse.masks import make_identity
    from contextlib import ExitStack

    F32 = mybir.dt.float32
    F32R = mybir.dt.float32r
    BF16 = mybir.dt.bfloat16
    AF = mybir.ActivationFunctionType
    OP = mybir.AluOpType

    nc = bacc.Bacc("TRN2", target_bir_lowering=False, debug=False, num_devices=R)

    xT_d = nc.dram_tensor("xT", [C, BN], BF16, kind="ExternalInput").ap()
    cT_d = nc.dram_tensor("cT", [C, BM], BF16, kind="ExternalInput").ap()
    mcol_d = nc.dram_tensor("mcol", [P, TM, 2], BF16, kind="ExternalInput").ap()
    wqkv_d = nc.dram_tensor("wqkv", [C, 3, DL], BF16, kind="ExternalInput").ap()
    wo_d = nc.dram_tensor("wo", [C, C], BF16, kind="ExternalInput").ap()
    bqk_d = nc.dram_tensor("bqk", [DL, 2], F32, kind="ExternalInput").ap()
    bo_d = nc.dram_tensor("bo", [P, C], F32, kind="ExternalInput").ap()
    out_d = nc.dram_tensor("out", [2, P, C], BF16, kind="ExternalOutput").ap()

    with tile.TileContext(nc) as tc, ExitStack() as es:
        const = es.enter_context(tc.tile_pool(name="const", bufs=1))
        kt_pool = es.enter_context(tc.tile_pool(name="kt", bufs=8))
        vn_pool = es.enter_context(tc.tile_pool(name="vn", bufs=8))
        qt_pool = es.enter_context(tc.tile_pool(name="qt", bufs=4))
        av_pool = es.enter_context(tc.tile_pool(name="av", bufs=2))
        agp = es.enter_context(tc.tile_pool(name="agp", bufs=2))
        outp = es.enter_context(tc.tile_pool(name="outp", bufs=4))
        dram = es.enter_context(tc.tile_pool(name="dram", bufs=1, space="DRAM"))
        pst = es.enter_context(tc.tile_pool(name="pst", bufs=2, space="PSUM"))
        psp = es.enter_context(tc.tile_pool(name="psp", bufs=2, space="PSUM"))
        pss = es.enter_context(tc.tile_pool(name="pss", bufs=2, space="PSUM"))
        psa = es.enter_context(tc.tile_pool(name="psa", bufs=2, space="PSUM"))

        # ---- constants: one priority-ordered stream on the sync queue so
        # early ctx slabs are never starved by later/larger loads ----
        wqkv_t = const.tile([P, CC, 3, DL], BF16, tag="wqkv")
        nc.sync.dma_start(wqkv_t[:], wqkv_d.rearrange("(cc p) w d -> p cc w d", p=P))
        wq_t = wqkv_t[:, :, 0]
        wk_t = wqkv_t[:, :, 1]
        wv_t = wqkv_t[:, :, 2]
        bqk_t = const.tile([P, 2], F32, tag="bqk")
        nc.sync.dma_start(bqk_t[:], bqk_d[:])
        bq_t = bqk_t[:, 0:1]
        bk_t = bqk_t[:, 1:2]
        mcolt = const.tile([P, TM, 2], BF16, tag="mcol")
        nc.sync.dma_start(mcolt[:], mcol_d[:])
        bo_t = const.tile([P, C], F32, tag="bo")
        nc.scalar.dma_start(bo_t[:], bo_d[:])
        identf = const.tile([P, P], F32, tag="idf")
        make_identity(nc, identf[:])
        identr = const.tile([P, P], F32R, tag="idr")
        nc.vector.tensor_copy(identr[:], identf[:])
        ones_b = const.tile([1, 64], BF16, tag="ones")
        nc.gpsimd.memset(ones_b[:], 1.0)
        wo_t = const.tile([P, CC, C], BF16, tag="wo")

        es2 = ExitStack()
        xn_pool = es2.enter_context(tc.tile_pool(name="xn", bufs=2))
        cn_pool = es2.enter_context(tc.tile_pool(name="cn", bufs=4))
        vt_pool = es2.enter_context(tc.tile_pool(name="vt", bufs=2))
        e_pool = es2.enter_context(tc.tile_pool(name="e", bufs=6))
        nrm_pool = es2.enter_context(tc.tile_pool(name="nrm", bufs=4))

        kt_tiles = []
        vn_tiles = []
        qt_tiles = []
        av_tiles = []
        a2a_ins = []
        a2a_outs = []
        cslabs = {}

        # ---- DMA stream, consumption order: ctx0 ctx1 x0 ctx2 x1 ctx3 x2
        # x3 ctx4.. (barrier + wo_t wedge in behind the early slabs) ----
        xslabs = []

        def x_dma(b):
            xs = xn_pool.tile([P, CC, 512], BF16, tag="xs")
            nc.sync.dma_start(
                xs[:], xT_d[:, b * 512:(b + 1) * 512]
                .rearrange("(cc p) n -> p cc n", p=P))
            xslabs.append(xs)

        def ctx_dma(mc, split=False):
            cs = cn_pool.tile([P, CC, 512], BF16, tag="cs")
            src_r = cT_d[:, mc * 512:(mc + 1) * 512].rearrange(
                "(cc p) n -> p cc n", p=P)
            if split:
                nc.sync.dma_start(cs[:, 0:4], src_r[:, 0:4])
                nc.sync.dma_start(cs[:, 4:8], src_r[:, 4:8])
            else:
                nc.sync.dma_start(cs[:], src_r)
            cslabs[mc] = cs

        ctx_dma(0, split=True)
        ctx_dma(1, split=True)
        x_dma(0)
        ctx_dma(2)
        ctx_dma(3)
        x_dma(1)

        bar_in = dram.tile([1, 4], F32, tag="barin")
        bar_out = dram.tile([R, 4], F32, tag="barout")
        nc.scalar.dma_start(bar_in[:], bqk_d.rearrange("d o -> o d")[0:1, 0:4])
        nc.gpsimd.collective_compute(
            "AllGather", OP.bypass, replica_groups=[list(range(R))],
            ins=[bar_in.opt()], outs=[bar_out.opt()])
        def q_proj(b):
            pq = psp.tile([P, 512], F32, tag="p")
            for cc in range(CC):
                nc.tensor.matmul(pq[:], lhsT=wq_t[:, cc], rhs=xslabs[b][:, cc, :],
                                 start=(cc == 0), stop=(cc == CC - 1))
            qt = qt_pool.tile([P, 512], BF16, tag="qt")
            nc.scalar.activation(qt[:], pq[:], AF.Identity, bias=bq_t, scale=1.0)
            qt_tiles.append(qt)

        # ---- Phase B slab chunks (emitted inline or as attention filler) ----
        def slab_chunks(mc):
            def c_k():
                pk = psp.tile([P, 512], F32, tag="p", name=f"pk{mc}")
                for cc in range(CC):
                    nc.tensor.matmul(pk[:], lhsT=wk_t[:, cc],
                                     rhs=cslabs[mc][:, cc, :],
                                     start=(cc == 0), stop=(cc == CC - 1))
                kt = kt_pool.tile([P, 512], BF16, tag="kt")
                nc.scalar.activation(kt[:], pk[:], AF.Identity, bias=bk_t, scale=1.0)
                kt_tiles.append(kt)

            def c_v():
                pv = psp.tile([P, 512], F32, tag="p", name=f"pv{mc}")
                for cc in range(CC):
                    nc.tensor.matmul(pv[:], lhsT=wv_t[:, cc],
                                     rhs=cslabs[mc][:, cc, :],
                                     start=(cc == 0), stop=(cc == CC - 1))
                vt_sb = vt_pool.tile([P, 512], F32R, tag="vt")
                nc.vector.tensor_copy(vt_sb[:], pv[:])
                cslabs[mc] = None
                return vt_sb

            def c_t(vt_sb):
                pvt = pst.tile([P, 512], F32R, tag="t")
                for j in range(4):
                    nc.tensor.transpose(pvt[:, j * P:(j + 1) * P],
                                        vt_sb[:, j * P:(j + 1) * P], identr[:])
                vn = vn_pool.tile([P, 4, 2, 65], BF16, tag="vn")
                nc.vector.tensor_copy(
                    vn[:, :, :, 0:64],
                    pvt[:].rearrange("p (j a d) -> p j a d", j=4, a=2))
                nc.vector.tensor_copy(vn[:, :, :, 64:65],
                                      mcolt[:, mc * 4:(mc + 1) * 4, :])
                vn_tiles.append(vn)

            state = {}
            def chunk1(): state['v'] = None; c_k()
            def chunk2(): state['v'] = c_v()
            def chunk3(): c_t(state['v'])
            return [chunk1, chunk2, chunk3]

        # ---- attention per batch, with filler chunks interleaved ----
        def attention_batch(b, filler):
            pav = [psa.tile([P, 512], F32, tag="a", name=f"pav{_h}") for _h in range(2)]
            fi = 0
            for mt in range(MT):
                tm = b * MT + mt
                mc, j = tm // 4, tm % 4
                kt = kt_tiles[mc]
                vn = vn_tiles[mc]
                ps = [pss.tile([P, 512], F32, tag="s", name=f"ps{_h}") for _h in range(2)]
                for h in range(2):
                    nc.tensor.matmul(ps[h][:],
                                     lhsT=kt[h * 64:(h + 1) * 64, j * P:(j + 1) * P],
                                     rhs=qt_tiles[b][h * 64:(h + 1) * 64, :],
                                     start=True, stop=True)
                ee = []
                for h in range(2):
                    e = e_pool.tile([P, 512], BF16, tag="e")
                    nc.scalar.activation(e[:], ps[h][:], AF.Exp,
                                         bias=0.0, scale=float(SCALE))
                    ee.append(e)
                if fi < len(filler):
                    filler[fi]()
                    fi += 1
                first, last = (mt == 0), (mt == MT - 1)
                for h in range(2):
                    # lhsT = [V_h | mask]: rows 0:64 = (EV)^T, row 64 = denom
                    nc.tensor.matmul(pav[h][0:65, :], lhsT=vn[:, j, h, :],
                                     rhs=ee[h][:], start=first, stop=last)
            while fi < len(filler):
                filler[fi]()
                fi += 1
            avt = av_pool.tile([P, 512], BF16, tag="av")
            for h in range(2):
                den = nrm_pool.tile([1, 512], F32, tag="den")
                nc.vector.tensor_copy(den[:], pav[h][64:65, :])
                rec = nrm_pool.tile([1, 512], F32, tag="rec")
                nc.vector.reciprocal_approx_fast(rec[:], den[:])
                rec_b = nrm_pool.tile([1, 512], BF16, tag="recb")
                nc.vector.tensor_copy(rec_b[:], rec[:])
                pb = psp.tile([P, 512], F32, tag="p")
                nc.tensor.matmul(pb[0:64, :], lhsT=ones_b[:], rhs=rec_b[:],
                                 start=True, stop=True)
                bc = nrm_pool.tile([64, 512], F32, tag="bc")
                nc.vector.tensor_copy(bc[:], pb[0:64, :])
                nc.vector.scalar_tensor_tensor(
                    out=avt[h * 64:(h + 1) * 64, :], in0=pav[h][0:64, :],
                    scalar=1.0, in1=bc[:], op0=OP.mult, op1=OP.mult)
            av_tiles.append(avt)
            hh = b // 2
            if b % 2 == 0:
                a2a_in = dram.tile([R, P, P], BF16, name=f"a2ai{hh}")
                a2a_ins.append(a2a_in)
            else:
                a2a_in = a2a_ins[hh]
            # stage this batch's 4 chunks now (slot j gets batch 2hh+j//4)
            for j in range(R):
                if j // 4 != b % 2:
                    continue
                nc.scalar.dma_start(a2a_in[j, :, :],
                                    avt[:, (j % 4) * P:(j % 4 + 1) * P])
            if b % 2 == 1:
                a2a_out = dram.tile([R, P, P], BF16, name=f"a2ao{hh}")
                nc.gpsimd.collective_compute(
                    "AllToAll", OP.bypass, replica_groups=[list(range(R))],
                    ins=[a2a_in.opt()], outs=[a2a_out.opt()])
                a2a_outs.append(a2a_out)

        def wo_chunks(h):
            """Deferred behind the a2a so the collective completes off the
            critical path (in-order engine queues)."""
            state = {}

            def load():
                agt = agp.tile([P, CC, P], BF16, name=f"agt{h}", tag="ag")
                src_r = a2a_outs[h].rearrange("i p n -> p i n")
                nc.scalar.dma_start(agt[:, 0:4], src_r[:, 0:4])
                nc.scalar.dma_start(agt[:, 4:8], src_r[:, 4:8])
                state['agt'] = agt

            def ch_chunk(ch):
                agt = state['agt']
                po = psp.tile([P, 512], F32, tag="p", name=f"po{h}_{ch}")
                for cc in range(CC):
                    nc.tensor.matmul(po[:], lhsT=agt[:, cc, :],
                                     rhs=wo_t[:, cc, ch * 512:(ch + 1) * 512],
                                     start=(cc == 0), stop=(cc == CC - 1))
                ob = outp.tile([P, 512], BF16, tag="ob", name=f"ob{h}_{ch}")
                nc.vector.scalar_tensor_tensor(
                    out=ob[:], in0=po[:], scalar=1.0,
                    in1=bo_t[:, ch * 512:(ch + 1) * 512],
                    op0=OP.mult, op1=OP.add)
                nc.scalar.dma_start(out_d[h, :, ch * 512:(ch + 1) * 512], ob[:])

            return [load, lambda: ch_chunk(0), lambda: ch_chunk(1)]

        # slabs 0-3 inline (batch 0's keys), then attention batches with the
        # next batch's slab chunks (or wo_half(0)) as PE filler.
        # prologue: slabs 0-3 + q-projections, ctx dmas rolling 4 ahead
        chunks0 = slab_chunks(0)
        chunks1 = slab_chunks(1)
        chunks0[0](); chunks0[1](); chunks0[2]()
        ctx_dma(4)
        q_proj(0)
        chunks1[0](); chunks1[1](); chunks1[2]()
        ctx_dma(5)
        q_proj(1)
        for c in slab_chunks(2):
            c()
        x_dma(2)
        ctx_dma(6)
        q_proj(2)
        for c in slab_chunks(3):
            c()
        x_dma(3)
        ctx_dma(7)
        q_proj(3)
        wo_dmaed = False
        noop = lambda: None
        for b in range(B):
            if b < 3:
                filler = []
                nslab = 4 if b < 2 else 3
                for k in range(nslab):
                    mc = 4 * (b + 1) + k
                    cks = slab_chunks(mc)
                    def mk_tail(ck3, nxt):
                        def f():
                            ck3()
                            if nxt is not None:
                                ctx_dma(nxt)
                        return f
                    nxt = mc + 4 if mc + 4 < 16 else None
                    filler += [cks[0], cks[1], mk_tail(cks[2], nxt)]
            else:
                s15 = slab_chunks(15)
                filler = [s15[0], s15[1], s15[2]]
            attention_batch(b, filler)
            if b == 3:
                # wo_half(0) runs while the h=1 AllToAll is in flight
                for c in wo_chunks(0):
                    c()
            if b == 2 and not wo_dmaed:
                # wo lands between the two AllToAlls (collectives bounce
                # through HBM; avoid competing with them for bandwidth)
                nc.sync.dma_start(wo_t[:], wo_d.rearrange("(cc p) c -> p cc c", p=P))
                wo_dmaed = True
        for c in wo_chunks(1):
            c()

        es2.close()

    nc.compile()
    return nc


def _get_nc():
    if "nc" not in _cached:
        _cached["nc"] = _build()
    return _cached["nc"]


def _prep_inputs(x, context, ctx_key_padding_mask, Wq, bq, Wk, bk, Wv, bv, Wo, bo):
    bf16 = ml_dtypes.bfloat16
    x = np.asarray(x, dtype=np.float32).reshape(BN, C)
    ctx = np.asarray(context, dtype=np.float32).reshape(BM, C)
    mask = np.asarray(ctx_key_padding_mask, dtype=np.int32)
    Wq = np.asarray(Wq, dtype=np.float32)
    Wk = np.asarray(Wk, dtype=np.float32)
    Wv = np.asarray(Wv, dtype=np.float32)
    Wo = np.asarray(Wo, dtype=np.float32)
    bq = np.asarray(bq, dtype=np.float32)
    bk = np.asarray(bk, dtype=np.float32)
    bv = np.asarray(bv, dtype=np.float32)
    bo = np.asarray(bo, dtype=np.float32)
    # bv folds through the (row-stochastic) attention and Wo exactly:
    # out = (attn + bv) @ Wo.T + bo = attn @ Wo.T + (bo + Wo @ bv)
    bo_eff = (bo.astype(np.float64) + Wo.astype(np.float64) @ bv.astype(np.float64)
              ).astype(np.float32)
    bo_bc = np.ascontiguousarray(np.broadcast_to(bo_eff, (P, C)))
    # key-padding mask folded into ctx^T: masked keys' columns become 0, so
    # v is masked for free; their k rows are bk-only but those E values are
    # multiplied by the zero mask column in both softmax sums.
    mf = (mask != 0).astype(np.float32).reshape(BM)      # [BM]
    xT = np.ascontiguousarray(x.T).astype(bf16)          # [C, BN]
    cT = np.ascontiguousarray((ctx * mf[:, None]).T).astype(bf16)  # [C, BM]
    # mcol[p, tm, h] = mask value for key tm*128+p (same for both heads)
    mcol = np.ascontiguousarray(
        np.broadcast_to(mf.reshape(TM, P).T[:, :, None], (P, TM, 2))).astype(bf16)
    wo_full = np.ascontiguousarray(Wo.T).astype(bf16)
    in_maps = []
    for r in range(R):
        sl = slice(r * DL, (r + 1) * DL)
        wqkv = np.stack([Wq[sl, :].T, Wk[sl, :].T, Wv[sl, :].T], axis=1)
        bqk = np.stack([bq[sl], bk[sl]], axis=1)
        in_maps.append({
            "xT": xT, "cT": cT, "mcol": mcol,
            "wqkv": np.ascontiguousarray(wqkv).astype(bf16),
            "wo": wo_full,
            "bqk": np.ascontiguousarray(bqk, dtype=np.float32),
            "bo": bo_bc,
        })
    return in_maps


def _run(in_maps, **kwargs):
    from concourse.bass_utils import run_bass_kernel_spmd
    nc = _get_nc()
    return run_bass_kernel_spmd(nc, in_maps, list(range(R)), **kwargs)


def kernel(x, context, ctx_key_padding_mask, Wq, bq, Wk, bk, Wv, bv, Wo, bo):
    in_maps = _prep_inputs(x, context, ctx_key_padding_mask,
                           Wq, bq, Wk, bk, Wv, bv, Wo, bo)
    res = _run(in_maps).results
    out = np.empty((BN, C), dtype=np.float32)
    for r in range(R):
        o = res[r]["out"]          # [2, 128, C]: half h -> batch 2h + r//4,
        for h in range(2):         # rows (r%4)*128 ...
            b = 2 * h + r // 4
            row = b * N + (r % 4) * P
            out[row:row + P] = o[h]
    return np.ascontiguousarray(out.reshape(B, N, C))
